# revision 22
# baseline (speedup 1.0000x reference)
"""Multi-head causal attention (Whisper-style) on 8 TRN2 NeuronCores.

Problem: B=4, T=2048, S=1024, H=16, D=64, fp32.

Sharding: core c = 2*b + g handles batch b (0..3) and head-group g (0..1,
8 heads = 512 channels). Each core computes its partial out-projection
(contraction over its 512 channels); the host sums the two partials per
batch and adds the output bias. No device collectives.

Per-core kernel (fp16 matmuls, fp32 PSUM accumulation; fp16 keeps
tf32-class relative precision here because every tensor is range-bounded):
all three stages are interleaved over t-groups of 512 so the PE stream
stays dense (HAM stays at K=8/8) and ACT's exp overlaps projection /
out-projection matmuls:

  for t in 0..3:
    stage A: qT/kT = (scale*W @ x^T) in [d, T] head-pair layout
             (partitions 0-63 head 2j, 64-127 head 2j+1); v in [T, d+1]
             layout with a ones column (PV then yields softmax sums L).
    stage B (attention for q-group t): per head-pair, for each k-chunk
             one shared PSUM pair-tile [128, 1024] holds S^T of both
             heads (K=64 matmuls packed in the PE via tile_position row
             strips); mask handled as deduplicated [128,128] block
             patterns (causal = 1 triangle tile) with fully-masked
             column prefixes trimmed from S/exp/PV; one ACT exp per
             k-chunk covers both heads; PV accumulates O^T[d+1, q];
             row 64 is L; normalize via DVE reciprocal_approx_fast +
             GPSIMD partition broadcast.
    stage C: partial out[q, s] = wv_norm @ Wo_slice^T for q-group t.
"""
import sys

import numpy as np

sys.path.insert(0, "/opt/trn_rl_repo")

import concourse.mybir as mybir
import concourse.tile as tile
from concourse import bacc
from concourse.bass_utils import run_bass_kernel_spmd

B, T, S, H, D = 4, 2048, 1024, 16, 64
N_CORES = 8
CH = S // 2          # 512 channels per core (8 heads)
HPC = H // 2 // 2    # 4 head pairs per core
KC = T // 128        # 16 k-chunks
QG = T // 512        # 4 q-groups / t-groups
SC = S // 128        # 8 s-chunks (contraction for projections)
SCALE = float(D) ** -0.25
MAX_PATTERNS = 16

_cache = {}


def _classify_mask(maskT: np.ndarray):
    """Classify maskT [k, q] on a (ki, qg) grid of [128, 512] tiles, each
    split into 4 [128, 128] column blocks.

    Returns (tiles, patterns):
      tiles[(ki, qg)] = None (tile fully masked)
                      | (cj, blocks, full) where cj = fully-masked column
                        prefix length, blocks = ((col_off, pat_idx), ...)
                        mask blocks to add, full = True -> add the whole
                        [128, 512] tile from maskT instead (fallback).
      patterns: list of [128, 128] float32 arrays (deduplicated).
    """
    pat_idx = {}
    patterns = []
    tiles = {}
    for ki in range(KC):
        for qg in range(QG):
            t = maskT[ki * 128 : (ki + 1) * 128, qg * 512 : (qg + 1) * 512]
            blocks_cls = []
            for c in range(4):
                blk = t[:, c * 128 : (c + 1) * 128]
                if np.all(blk <= -1e8):
                    blocks_cls.append(("skip", blk))
                elif np.all(blk == 0.0):
                    blocks_cls.append(("zero", blk))
                else:
                    blocks_cls.append(("mixed", blk))
            j = 0
            while j < 4 and blocks_cls[j][0] == "skip":
                j += 1
            if j == 4:
                tiles[(ki, qg)] = None
                continue
            blks = []
            for c in range(j, 4):
                cls_c, blk = blocks_cls[c]
                if cls_c == "zero":
                    continue
                key = blk.tobytes()
                if key not in pat_idx:
                    pat_idx[key] = len(patterns)
                    patterns.append(np.ascontiguousarray(blk))
                blks.append((c * 128, pat_idx[key]))
            tiles[(ki, qg)] = (j * 128, tuple(blks), False)
    if len(patterns) > MAX_PATTERNS:
        # fallback: whole-tile mask adds straight from maskT
        patterns = []
        for k in tiles:
            if tiles[k] is not None:
                cj, blks, _ = tiles[k]
                tiles[k] = (0, (), bool(blks))
    return tiles, patterns


def _build(tiles, n_pat):
    f32, f16 = mybir.dt.float32, mybir.dt.float16
    nc = bacc.Bacc(trn_type="TRN2", target_bir_lowering=False, debug=False)
    NP = max(n_pat, 1)

    xT_d = nc.dram_tensor("xT", [S, T], f16, kind="ExternalInput")
    wqT_d = nc.dram_tensor("wqT", [S, CH], f16, kind="ExternalInput")
    wkT_d = nc.dram_tensor("wkT", [S, CH], f16, kind="ExternalInput")
    wvT_d = nc.dram_tensor("wvT", [S, CH], f16, kind="ExternalInput")
    woT_d = nc.dram_tensor("woT", [CH, S], f16, kind="ExternalInput")
    bq_d = nc.dram_tensor("bq", [CH], f32, kind="ExternalInput")
    bv_d = nc.dram_tensor("bv", [CH], f32, kind="ExternalInput")
    mpat_d = nc.dram_tensor("mpat", [NP, 128, 128], f32, kind="ExternalInput")
    maskT_d = nc.dram_tensor("maskT", [T, T], f32, kind="ExternalInput")
    out_d = nc.dram_tensor("out", [T, S], f32, kind="ExternalOutput")

    with tile.TileContext(nc) as tc:
        with (
            tc.tile_pool(name="persist", bufs=1) as pp,
            tc.tile_pool(name="xtp", bufs=2) as xtp,
            tc.tile_pool(name="ptp", bufs=8) as ptp,
            tc.tile_pool(name="mfp", bufs=4) as mfp,
            tc.tile_pool(name="lp", bufs=2) as lp,
            tc.tile_pool(name="op", bufs=2) as op,
            tc.tile_pool(name="psum", bufs=3, space="PSUM") as ps,
            tc.tile_pool(name="psum_o", bufs=1, space="PSUM") as ps_o,
        ):
            # weights / biases / patterns, resident
            wq_t = pp.tile([128, SC, CH], f16)
            wk_t = pp.tile([128, SC, CH], f16)
            wv_t = pp.tile([128, SC, CH], f16)
            wo_t = pp.tile([128, HPC, S], f16)
            bq_t = pp.tile([128, HPC], f32)
            bvb_t = pp.tile([128, CH], f32)
            bv1_t = pp.tile([1, CH], f32)
            pat_t = pp.tile([128, NP, 128], f32)
            wq_r = wqT_d.rearrange("(o p) c -> p o c", p=128)
            nc.sync.dma_start(wq_t[:, :1], wq_r[:, :1])
            nc.sync.dma_start(wq_t[:, 1:2], wq_r[:, 1:2])
            nc.sync.dma_start(bq_t[:], bq_d.rearrange("(o p) -> p o", p=128))
            nc.sync.dma_start(bv1_t[:], bv_d[None, :])
            nc.gpsimd.partition_broadcast(bvb_t[:], bv1_t[:])

            # per-t-group activations
            qT_ts, kT_ts, v_ts, wvT_ts = [], [], [], []
            for tg in range(QG):
                qT_ts.append(pp.tile([128, HPC, 512], f16, name=f"qT{tg}"))
                kT_ts.append(pp.tile([128, HPC, 512], f16, name=f"kT{tg}"))
                v_ts.append(pp.tile([128, 4, 8, D + 1], f16, name=f"v{tg}"))
                wvT_ts.append(pp.tile([128, HPC, 512], f16, name=f"wv{tg}"))
                nc.vector.memset(v_ts[tg][:, :, :, D], 1.0)

            def stage_A_units(tg):
                # ---------- stage A: projections for t-group tg ----------
                units = []
                xt = xtp.tile([128, SC, 512], f16, tag="xt")
                xt_r = xT_d[:, tg * 512 : (tg + 1) * 512].rearrange(
                    "(o p) t -> p o t", p=128
                )
                nc.sync.dma_start(xt[:, :1], xt_r[:, :1])
                nc.sync.dma_start(xt[:, 1:2], xt_r[:, 1:2])
                nc.sync.dma_start(xt[:, 2:4], xt_r[:, 2:4])
                nc.sync.dma_start(xt[:, 4:6], xt_r[:, 4:6])
                nc.sync.dma_start(xt[:, 6:], xt_r[:, 6:])
                if tg == 0:
                    # deferred resident loads, ordered by first use
                    nc.sync.dma_start(wq_t[:, 2:], wq_r[:, 2:])
                    nc.sync.dma_start(
                        wk_t[:], wkT_d.rearrange("(o p) c -> p o c", p=128)
                    )
                    nc.sync.dma_start(
                        wv_t[:], wvT_d.rearrange("(o p) c -> p o c", p=128)
                    )
                    nc.sync.dma_start(pat_t[:], mpat_d.rearrange("n p c -> p n c"))
                    nc.sync.dma_start(
                        wo_t[:], woT_d.rearrange("(o p) s -> p o s", p=128)
                    )
                def unit_dch(dch):
                    csl = slice(dch * 128, (dch + 1) * 128)
                    spq = ps.tile([128, 1024], f32, tag="sp", name=f"qk{tg}{dch}")
                    for s in range(SC):
                        nc.tensor.matmul(
                            spq[:, :512], wq_t[:, s, csl], xt[:, s],
                            start=(s == 0), stop=(s == SC - 1),
                        )
                    for s in range(SC):
                        nc.tensor.matmul(
                            spq[:, 512:], wk_t[:, s, csl], xt[:, s],
                            start=(s == 0), stop=(s == SC - 1),
                        )
                    nc.scalar.activation(
                        qT_ts[tg][:, dch], spq[:, :512],
                        mybir.ActivationFunctionType.Identity,
                        bias=bq_t[:, dch : dch + 1],
                    )
                    nc.scalar.copy(kT_ts[tg][:, dch], spq[:, 512:])

                def unit_v(tp2):
                    spv = ps.tile([128, 1024], f32, tag="sp", name=f"v{tg}{tp2}")
                    for half in range(2):
                        ti = tp2 * 2 + half
                        hsl = slice(half * 512, (half + 1) * 512)
                        for s in range(SC):
                            nc.tensor.matmul(
                                spv[:, hsl],
                                xt[:, s, ti * 128 : (ti + 1) * 128],
                                wv_t[:, s],
                                start=(s == 0), stop=(s == SC - 1),
                            )
                    for half in range(2):
                        ti = tp2 * 2 + half
                        nc.vector.tensor_add(
                            v_ts[tg][:, ti, :, :D],
                            spv[:, half * 512 : (half + 1) * 512].rearrange(
                                "p (h d) -> p h d", d=D
                            ),
                            bvb_t[:].rearrange("p (h d) -> p h d", d=D),
                        )

                for dch in range(HPC):
                    units.append(lambda dch=dch: unit_dch(dch))
                for tp2 in range(2):
                    units.append(lambda tp2=tp2: unit_v(tp2))
                return units

            def stage_B_units(qg, c_prev=None):
                # ---------- stage B: attention for q-group qg ----------
                active = [ki for ki in range(KC) if tiles[(ki, qg)] is not None]
                units = []

                def unit_hp(hp):
                    oTs = [
                        ps_o.tile([D + 1, 512], f32, tag=f"o{par}",
                                  name=f"o{qg}{hp}{par}")
                        for par in range(2)
                    ]
                    n_act = len(active)
                    pending = {}

                    def emit_S(i):
                        ki = active[i]
                        cj, blks, full = tiles[(ki, qg)]
                        tgk, tik = ki // 4, ki % 4
                        sp = ps.tile([128, 1024], f32, tag="sp",
                                     name=f"s{qg}{hp}{ki}")
                        for par in range(2):
                            p0 = par * 64
                            nc.tensor.matmul(
                                sp[:, par * 512 + cj : (par + 1) * 512],
                                kT_ts[tgk][p0 : p0 + 64, hp,
                                           tik * 128 : (tik + 1) * 128],
                                qT_ts[qg][p0 : p0 + 64, hp, cj:],
                                start=True, stop=True,
                                tile_position=(p0, 0),
                            )
                        if full:
                            mf = mfp.tile([128, 512], f32, tag="mf")
                            nc.sync.dma_start(
                                mf[:],
                                maskT_d[ki * 128 : (ki + 1) * 128,
                                        qg * 512 : (qg + 1) * 512],
                            )
                            for par in range(2):
                                nc.vector.tensor_add(
                                    sp[:, par * 512 : (par + 1) * 512],
                                    sp[:, par * 512 : (par + 1) * 512],
                                    mf[:],
                                )
                        else:
                            spv2 = sp[:].rearrange("p (two q) -> p two q", two=2)
                            for cb, pi in blks:
                                nc.vector.tensor_add(
                                    spv2[:, :, cb : cb + 128],
                                    spv2[:, :, cb : cb + 128],
                                    pat_t[:, pi, None, :].to_broadcast(
                                        (128, 2, 128)
                                    ),
                                )
                        pT = ptp.tile([128, 1024], f16, tag="pT",
                                      name=f"p{qg}{hp}{ki}")
                        nc.scalar.activation(
                            pT[:].rearrange("p (two q) -> p two q", two=2)[:, :, cj:],
                            sp[:].rearrange("p (two q) -> p two q", two=2)[:, :, cj:],
                            mybir.ActivationFunctionType.Exp,
                        )
                        pending[i] = (pT, cj)

                    def emit_PV(i):
                        ki = active[i]
                        tgk, tik = ki // 4, ki % 4
                        pT, cj = pending.pop(i)
                        for par in range(2):
                            h = hp * 2 + par
                            nc.tensor.matmul(
                                oTs[par][:, cj:],
                                v_ts[tgk][:, tik, h],
                                pT[:, par * 512 + cj : (par + 1) * 512],
                                start=(i == 0), stop=(i == n_act - 1),
                                skip_group_check=True,
                            )

                    LAG = 2
                    for i in range(n_act + LAG):
                        if i < n_act:
                            emit_S(i)
                        if i - LAG >= 0:
                            emit_PV(i - LAG)

                    for par in range(2):
                        p0 = par * 64
                        lrow = lp.tile([1, 512], f32, tag="lr")
                        nc.vector.tensor_copy(lrow[:], oTs[par][D : D + 1, :])
                        lrec = lp.tile([1, 512], f32, tag="lrec")
                        nc.vector.reciprocal_approx_fast(lrec[:], lrow[:])
                        lb = lp.tile([64, 512], f32, tag=f"lb{par}")
                        nc.gpsimd.partition_broadcast(lb[:], lrec[:])
                        nc.vector.tensor_mul(
                            wvT_ts[qg][p0 : p0 + 64, hp],
                            oTs[par][:D, :],
                            lb[:],
                        )
                    if c_prev is not None:
                        stage_C_chunk(c_prev, hp)

                for hp in range(HPC):
                    units.append(lambda hp=hp: unit_hp(hp))
                return units

            def stage_C_chunk(qg, qc):
                # ---- stage C: out projection for q-group qg, chunk qc ----
                osb = op.tile([128, S], f32, tag="osb", name=f"ou{qg}{qc}")
                sp3 = ps.tile([128, 1024], f32, tag="sp", name=f"po{qg}{qc}")
                for sh in range(2):
                    for co in range(HPC):
                        nc.tensor.matmul(
                            sp3[:, sh * 512 : (sh + 1) * 512],
                            wvT_ts[qg][:, co, qc * 128 : (qc + 1) * 128],
                            wo_t[:, co, sh * 512 : (sh + 1) * 512],
                            start=(co == 0), stop=(co == HPC - 1),
                        )
                nc.vector.tensor_copy(osb[:], sp3[:])
                nc.sync.dma_start(
                    out_d[qg * 512 + qc * 128 : qg * 512 + (qc + 1) * 128, :],
                    osb[:],
                )

            for u in stage_A_units(0):
                u()
            for tg in (1, 2):
                au = stage_A_units(tg)
                bu = stage_B_units(tg - 1, c_prev=tg - 2 if tg >= 2 else None)
                for i in range(HPC):
                    au[i]()
                    bu[i]()
                au[HPC]()
                au[HPC + 1]()
            for u in stage_A_units(3):
                u()
            b2 = stage_B_units(2, c_prev=1)
            b3 = stage_B_units(3, c_prev=None)
            for i in range(HPC):
                b2[i]()
                b3[i]()
            for qc in range(4):
                stage_C_chunk(2, qc)
            for qc in range(4):
                stage_C_chunk(3, qc)

    nc.compile()
    return nc


def _tiles_key(tiles, n_pat):
    return (n_pat, tuple(sorted(
        (k, v if v is None else (v[0], v[1], v[2])) for k, v in tiles.items()
    )))


def prepare(x, mask, Wq, bq, Wk, Wv, bv, Wo, bo):
    """Build (or fetch cached) the compiled Bass module and the per-core
    input maps for the given full inputs."""
    x = np.asarray(x, dtype=np.float32)
    mask = np.asarray(mask, dtype=np.float32)
    Wq = np.asarray(Wq, dtype=np.float32)
    bq = np.asarray(bq, dtype=np.float32)
    Wk = np.asarray(Wk, dtype=np.float32)
    Wv = np.asarray(Wv, dtype=np.float32)
    bv = np.asarray(bv, dtype=np.float32)
    Wo = np.asarray(Wo, dtype=np.float32)
    bo = np.asarray(bo, dtype=np.float32)

    maskT = np.ascontiguousarray(mask.T)
    tiles, patterns = _classify_mask(maskT)
    n_pat = len(patterns)
    key = _tiles_key(tiles, n_pat)
    if key not in _cache:
        _cache[key] = _build(tiles, n_pat)
    nc = _cache[key]

    NP = max(n_pat, 1)
    mpat = np.zeros((NP, 128, 128), dtype=np.float32)
    for i, p in enumerate(patterns):
        mpat[i] = p

    in_maps = []
    for c in range(N_CORES):
        b, g = c // 2, c % 2
        chsl = slice(g * CH, (g + 1) * CH)
        in_maps.append(
            {
                "xT": np.ascontiguousarray(x[b].T.astype(np.float16)),
                "wqT": np.ascontiguousarray((SCALE * Wq[chsl]).T.astype(np.float16)),
                "wkT": np.ascontiguousarray((SCALE * Wk[chsl]).T.astype(np.float16)),
                "wvT": np.ascontiguousarray(Wv[chsl].T.astype(np.float16)),
                "woT": np.ascontiguousarray(Wo[:, chsl].T.astype(np.float16)),
                "bq": np.ascontiguousarray(SCALE * bq[chsl]),
                "bv": np.ascontiguousarray(bv[chsl]),
                "mpat": mpat,
                "maskT": maskT,
            }
        )

    return nc, in_maps


def kernel(x, mask, Wq, bq, Wk, Wv, bv, Wo, bo):
    nc, in_maps = prepare(x, mask, Wq, bq, Wk, Wv, bv, Wo, bo)
    res = run_bass_kernel_spmd(nc, in_maps, core_ids=list(range(N_CORES)))
    out = np.empty((B, T, S), dtype=np.float32)
    for b in range(B):
        out[b] = res.results[2 * b]["out"] + res.results[2 * b + 1]["out"]
    out += np.asarray(bo, dtype=np.float32)
    return out


# revision 24
# speedup vs baseline: 1.0520x; 1.0520x over previous
"""Multi-head causal attention (Whisper-style) on 8 TRN2 NeuronCores.

Problem: B=4, T=2048, S=1024, H=16, D=64, fp32.

Sharding: core c = 2*b + g handles batch b (0..3) and head-group g (0..1,
8 heads = 512 channels). Each core computes its partial out-projection
(contraction over its 512 channels); the host sums the two partials per
batch and adds the output bias. No device collectives.

Per-core kernel (fp16 matmuls, fp32 PSUM accumulation; fp16 keeps
tf32-class relative precision here because every tensor is range-bounded):
all three stages are interleaved over t-groups of 512 so the PE stream
stays dense (HAM stays at K=8/8) and ACT's exp overlaps projection /
out-projection matmuls:

  for t in 0..3:
    stage A: qT/kT = (scale*W @ x^T) in [d, T] head-pair layout
             (partitions 0-63 head 2j, 64-127 head 2j+1); v in [T, d+1]
             layout with a ones column (PV then yields softmax sums L).
    stage B (attention for q-group t): per head-pair, for each k-chunk
             one shared PSUM pair-tile [128, 1024] holds S^T of both
             heads (K=64 matmuls packed in the PE via tile_position row
             strips); mask handled as deduplicated [128,128] block
             patterns (causal = 1 triangle tile) with fully-masked
             column prefixes trimmed from S/exp/PV; one ACT exp per
             k-chunk covers both heads; PV accumulates O^T[d+1, q];
             row 64 is L; normalize via DVE reciprocal_approx_fast +
             GPSIMD partition broadcast.
    stage C: partial out[q, s] = wv_norm @ Wo_slice^T for q-group t.
"""
import sys

import numpy as np

sys.path.insert(0, "/opt/trn_rl_repo")

import concourse.mybir as mybir
import concourse.tile as tile
from concourse import bacc
from concourse.bass_utils import run_bass_kernel_spmd

B, T, S, H, D = 4, 2048, 1024, 16, 64
N_CORES = 8
CH = S // 2          # 512 channels per core (8 heads)
HPC = H // 2 // 2    # 4 head pairs per core
KC = T // 128        # 16 k-chunks
QG = T // 512        # 4 q-groups / t-groups
SC = S // 128        # 8 s-chunks (contraction for projections)
SCALE = float(D) ** -0.25
MAX_PATTERNS = 16

_cache = {}


def _classify_mask(maskT: np.ndarray):
    """Classify maskT [k, q] on a (ki, qg) grid of [128, 512] tiles, each
    split into 4 [128, 128] column blocks.

    Returns (tiles, patterns):
      tiles[(ki, qg)] = None (tile fully masked)
                      | (cj, blocks, full) where cj = fully-masked column
                        prefix length, blocks = ((col_off, pat_idx), ...)
                        mask blocks to add, full = True -> add the whole
                        [128, 512] tile from maskT instead (fallback).
      patterns: list of [128, 128] float32 arrays (deduplicated).
    """
    pat_idx = {}
    patterns = []
    tiles = {}
    for ki in range(KC):
        for qg in range(QG):
            t = maskT[ki * 128 : (ki + 1) * 128, qg * 512 : (qg + 1) * 512]
            blocks_cls = []
            for c in range(4):
                blk = t[:, c * 128 : (c + 1) * 128]
                if np.all(blk <= -1e8):
                    blocks_cls.append(("skip", blk))
                elif np.all(blk == 0.0):
                    blocks_cls.append(("zero", blk))
                else:
                    blocks_cls.append(("mixed", blk))
            j = 0
            while j < 4 and blocks_cls[j][0] == "skip":
                j += 1
            if j == 4:
                tiles[(ki, qg)] = None
                continue
            blks = []
            for c in range(j, 4):
                cls_c, blk = blocks_cls[c]
                if cls_c == "zero":
                    continue
                key = blk.tobytes()
                if key not in pat_idx:
                    pat_idx[key] = len(patterns)
                    patterns.append(np.ascontiguousarray(blk))
                blks.append((c * 128, pat_idx[key]))
            tiles[(ki, qg)] = (j * 128, tuple(blks), False)
    if len(patterns) > MAX_PATTERNS:
        # fallback: whole-tile mask adds straight from maskT
        patterns = []
        for k in tiles:
            if tiles[k] is not None:
                cj, blks, _ = tiles[k]
                tiles[k] = (0, (), bool(blks))
    return tiles, patterns


def _build(tiles, n_pat):
    f32, f16 = mybir.dt.float32, mybir.dt.float16
    nc = bacc.Bacc(trn_type="TRN2", target_bir_lowering=False, debug=False)
    NP = max(n_pat, 1)

    xT_d = nc.dram_tensor("xT", [S, T], f16, kind="ExternalInput")
    wqT_d = nc.dram_tensor("wqT", [S, CH], f16, kind="ExternalInput")
    wkT_d = nc.dram_tensor("wkT", [S, CH], f16, kind="ExternalInput")
    wvT_d = nc.dram_tensor("wvT", [S, CH], f16, kind="ExternalInput")
    woT_d = nc.dram_tensor("woT", [CH, S], f16, kind="ExternalInput")
    bq_d = nc.dram_tensor("bq", [CH], f32, kind="ExternalInput")
    bv_d = nc.dram_tensor("bv", [CH], f32, kind="ExternalInput")
    mpat_d = nc.dram_tensor("mpat", [NP, 128, 128], f32, kind="ExternalInput")
    maskT_d = nc.dram_tensor("maskT", [T, T], f32, kind="ExternalInput")
    out_d = nc.dram_tensor("out", [T, S], f32, kind="ExternalOutput")

    with tile.TileContext(nc) as tc:
        with (
            tc.tile_pool(name="persist", bufs=1) as pp,
            tc.tile_pool(name="xtp", bufs=2) as xtp,
            tc.tile_pool(name="ptp", bufs=8) as ptp,
            tc.tile_pool(name="mfp", bufs=4) as mfp,
            tc.tile_pool(name="lp", bufs=4) as lp,
            tc.tile_pool(name="op", bufs=3) as op,
            tc.tile_pool(name="psum", bufs=3, space="PSUM") as ps,
            tc.tile_pool(name="psum_o", bufs=1, space="PSUM") as ps_o,
        ):
            # weights / biases / patterns, resident
            wq_t = pp.tile([128, SC, CH], f16)
            wk_t = pp.tile([128, SC, CH], f16)
            wv_t = pp.tile([128, SC, CH], f16)
            wo_t = pp.tile([128, HPC, S], f16)
            bq_t = pp.tile([128, HPC], f32)
            bvb_t = pp.tile([128, CH], f32)
            bv1_t = pp.tile([1, CH], f32)
            pat_t = pp.tile([128, NP, 128], f32)
            wq_r = wqT_d.rearrange("(o p) c -> p o c", p=128)
            nc.sync.dma_start(wq_t[:, :1], wq_r[:, :1])
            nc.sync.dma_start(wq_t[:, 1:2], wq_r[:, 1:2])
            nc.sync.dma_start(bq_t[:], bq_d.rearrange("(o p) -> p o", p=128))
            nc.sync.dma_start(bv1_t[:], bv_d[None, :])
            nc.gpsimd.partition_broadcast(bvb_t[:], bv1_t[:])

            # per-t-group activations
            qT_ts, kT_ts, v_ts, wvT_ts = [], [], [], []
            for tg in range(QG):
                qT_ts.append(pp.tile([128, HPC, 512], f16, name=f"qT{tg}"))
                kT_ts.append(pp.tile([128, HPC, 512], f16, name=f"kT{tg}"))
                v_ts.append(pp.tile([128, 4, 8, D + 1], f16, name=f"v{tg}"))
                wvT_ts.append(pp.tile([128, HPC, 512], f16, name=f"wv{tg}"))
                nc.vector.memset(v_ts[tg][:, :, :, D], 1.0)

            def stage_A_units(tg):
                # ---------- stage A: projections for t-group tg ----------
                units = []
                xt = xtp.tile([128, SC, 512], f16, tag="xt")
                xt_r = xT_d[:, tg * 512 : (tg + 1) * 512].rearrange(
                    "(o p) t -> p o t", p=128
                )
                nc.sync.dma_start(xt[:, :1], xt_r[:, :1])
                nc.sync.dma_start(xt[:, 1:2], xt_r[:, 1:2])
                nc.sync.dma_start(xt[:, 2:4], xt_r[:, 2:4])
                nc.sync.dma_start(xt[:, 4:6], xt_r[:, 4:6])
                nc.sync.dma_start(xt[:, 6:], xt_r[:, 6:])
                if tg == 0:
                    # deferred resident loads, ordered by first use
                    nc.sync.dma_start(wq_t[:, 2:], wq_r[:, 2:])
                    nc.sync.dma_start(
                        wk_t[:], wkT_d.rearrange("(o p) c -> p o c", p=128)
                    )
                    nc.sync.dma_start(
                        wv_t[:], wvT_d.rearrange("(o p) c -> p o c", p=128)
                    )
                    nc.sync.dma_start(pat_t[:], mpat_d.rearrange("n p c -> p n c"))
                    nc.sync.dma_start(
                        wo_t[:], woT_d.rearrange("(o p) s -> p o s", p=128)
                    )
                def unit_dch(dch):
                    csl = slice(dch * 128, (dch + 1) * 128)
                    spq = ps.tile([128, 1024], f32, tag="sp", name=f"qk{tg}{dch}")
                    for s in range(SC):
                        nc.tensor.matmul(
                            spq[:, :512], wq_t[:, s, csl], xt[:, s],
                            start=(s == 0), stop=(s == SC - 1),
                        )
                    for s in range(SC):
                        nc.tensor.matmul(
                            spq[:, 512:], wk_t[:, s, csl], xt[:, s],
                            start=(s == 0), stop=(s == SC - 1),
                        )
                    nc.scalar.activation(
                        qT_ts[tg][:, dch], spq[:, :512],
                        mybir.ActivationFunctionType.Identity,
                        bias=bq_t[:, dch : dch + 1],
                    )
                    nc.scalar.copy(kT_ts[tg][:, dch], spq[:, 512:])

                def unit_v(tp2):
                    spv = ps.tile([128, 1024], f32, tag="sp", name=f"v{tg}{tp2}")
                    for half in range(2):
                        ti = tp2 * 2 + half
                        hsl = slice(half * 512, (half + 1) * 512)
                        for s in range(SC):
                            nc.tensor.matmul(
                                spv[:, hsl],
                                xt[:, s, ti * 128 : (ti + 1) * 128],
                                wv_t[:, s],
                                start=(s == 0), stop=(s == SC - 1),
                            )
                    for half in range(2):
                        ti = tp2 * 2 + half
                        nc.vector.tensor_add(
                            v_ts[tg][:, ti, :, :D],
                            spv[:, half * 512 : (half + 1) * 512].rearrange(
                                "p (h d) -> p h d", d=D
                            ),
                            bvb_t[:].rearrange("p (h d) -> p h d", d=D),
                        )

                for dch in range(HPC):
                    units.append(lambda dch=dch: unit_dch(dch))
                for tp2 in range(2):
                    units.append(lambda tp2=tp2: unit_v(tp2))
                return units

            def stage_B_units(qg, c_prev=None):
                # ---------- stage B: attention for q-group qg ----------
                active = [ki for ki in range(KC) if tiles[(ki, qg)] is not None]
                units = []

                def unit_hp(hp):
                    oTs = [
                        ps_o.tile([D + 1, 512], f32, tag=f"o{par}",
                                  name=f"o{qg}{hp}{par}")
                        for par in range(2)
                    ]
                    n_act = len(active)
                    pending = {}

                    def emit_S(i):
                        ki = active[i]
                        cj, blks, full = tiles[(ki, qg)]
                        tgk, tik = ki // 4, ki % 4
                        sp = ps.tile([128, 1024], f32, tag="sp",
                                     name=f"s{qg}{hp}{ki}")
                        for par in range(2):
                            p0 = par * 64
                            nc.tensor.matmul(
                                sp[:, par * 512 + cj : (par + 1) * 512],
                                kT_ts[tgk][p0 : p0 + 64, hp,
                                           tik * 128 : (tik + 1) * 128],
                                qT_ts[qg][p0 : p0 + 64, hp, cj:],
                                start=True, stop=True,
                                tile_position=(p0, 0),
                            )
                        if full:
                            mf = mfp.tile([128, 512], f32, tag="mf")
                            nc.sync.dma_start(
                                mf[:],
                                maskT_d[ki * 128 : (ki + 1) * 128,
                                        qg * 512 : (qg + 1) * 512],
                            )
                            for par in range(2):
                                nc.vector.tensor_add(
                                    sp[:, par * 512 : (par + 1) * 512],
                                    sp[:, par * 512 : (par + 1) * 512],
                                    mf[:],
                                )
                        else:
                            spv2 = sp[:].rearrange("p (two q) -> p two q", two=2)
                            for cb, pi in blks:
                                nc.vector.tensor_add(
                                    spv2[:, :, cb : cb + 128],
                                    spv2[:, :, cb : cb + 128],
                                    pat_t[:, pi, None, :].to_broadcast(
                                        (128, 2, 128)
                                    ),
                                )
                        pT = ptp.tile([128, 1024], f16, tag="pT",
                                      name=f"p{qg}{hp}{ki}")
                        nc.scalar.activation(
                            pT[:].rearrange("p (two q) -> p two q", two=2)[:, :, cj:],
                            sp[:].rearrange("p (two q) -> p two q", two=2)[:, :, cj:],
                            mybir.ActivationFunctionType.Exp,
                        )
                        pending[i] = (pT, cj)

                    def emit_PV(i):
                        ki = active[i]
                        tgk, tik = ki // 4, ki % 4
                        pT, cj = pending.pop(i)
                        for par in range(2):
                            h = hp * 2 + par
                            nc.tensor.matmul(
                                oTs[par][:, cj:],
                                v_ts[tgk][:, tik, h],
                                pT[:, par * 512 + cj : (par + 1) * 512],
                                start=(i == 0), stop=(i == n_act - 1),
                                skip_group_check=True,
                            )

                    LAG = 2
                    for i in range(n_act + LAG):
                        if i < n_act:
                            emit_S(i)
                        if i - LAG >= 0:
                            emit_PV(i - LAG)

                    for par in range(2):
                        p0 = par * 64
                        lrow = lp.tile([1, 512], f32, tag="lr")
                        nc.vector.tensor_copy(lrow[:], oTs[par][D : D + 1, :])
                        lrec = lp.tile([1, 512], f32, tag="lrec")
                        nc.vector.reciprocal_approx_fast(lrec[:], lrow[:])
                        lb = lp.tile([64, 512], f32, tag=f"lb{par}")
                        nc.gpsimd.partition_broadcast(lb[:], lrec[:])
                        nc.vector.tensor_mul(
                            wvT_ts[qg][p0 : p0 + 64, hp],
                            oTs[par][:D, :],
                            lb[:],
                        )
                    if c_prev is not None:
                        stage_C_chunk(c_prev, hp)

                for hp in range(HPC):
                    units.append(lambda hp=hp: unit_hp(hp))
                return units

            def stage_C_chunk(qg, qc):
                # ---- stage C: out projection for q-group qg, chunk qc ----
                osb = op.tile([128, S], f32, tag="osb", name=f"ou{qg}{qc}")
                sp3 = ps.tile([128, 1024], f32, tag="sp", name=f"po{qg}{qc}")
                for sh in range(2):
                    for co in range(HPC):
                        nc.tensor.matmul(
                            sp3[:, sh * 512 : (sh + 1) * 512],
                            wvT_ts[qg][:, co, qc * 128 : (qc + 1) * 128],
                            wo_t[:, co, sh * 512 : (sh + 1) * 512],
                            start=(co == 0), stop=(co == HPC - 1),
                        )
                nc.vector.tensor_copy(osb[:], sp3[:])
                nc.sync.dma_start(
                    out_d[qg * 512 + qc * 128 : qg * 512 + (qc + 1) * 128, :],
                    osb[:],
                )

            for u in stage_A_units(0):
                u()
            for tg in range(1, QG):
                au = stage_A_units(tg)
                bu = stage_B_units(tg - 1, c_prev=tg - 2 if tg >= 2 else None)
                for i in range(HPC):
                    au[i]()
                    bu[i]()
                au[HPC]()
                au[HPC + 1]()
            for u in stage_B_units(QG - 1, c_prev=QG - 2):
                u()
            for qc in range(4):
                stage_C_chunk(QG - 1, qc)

    nc.compile()
    return nc


def _tiles_key(tiles, n_pat):
    return (n_pat, tuple(sorted(
        (k, v if v is None else (v[0], v[1], v[2])) for k, v in tiles.items()
    )))


def prepare(x, mask, Wq, bq, Wk, Wv, bv, Wo, bo):
    """Build (or fetch cached) the compiled Bass module and the per-core
    input maps for the given full inputs."""
    x = np.asarray(x, dtype=np.float32)
    mask = np.asarray(mask, dtype=np.float32)
    Wq = np.asarray(Wq, dtype=np.float32)
    bq = np.asarray(bq, dtype=np.float32)
    Wk = np.asarray(Wk, dtype=np.float32)
    Wv = np.asarray(Wv, dtype=np.float32)
    bv = np.asarray(bv, dtype=np.float32)
    Wo = np.asarray(Wo, dtype=np.float32)
    bo = np.asarray(bo, dtype=np.float32)

    maskT = np.ascontiguousarray(mask.T)
    tiles, patterns = _classify_mask(maskT)
    n_pat = len(patterns)
    key = _tiles_key(tiles, n_pat)
    if key not in _cache:
        _cache[key] = _build(tiles, n_pat)
    nc = _cache[key]

    NP = max(n_pat, 1)
    mpat = np.zeros((NP, 128, 128), dtype=np.float32)
    for i, p in enumerate(patterns):
        mpat[i] = p

    in_maps = []
    for c in range(N_CORES):
        b, g = c // 2, c % 2
        chsl = slice(g * CH, (g + 1) * CH)
        in_maps.append(
            {
                "xT": np.ascontiguousarray(x[b].T.astype(np.float16)),
                "wqT": np.ascontiguousarray((SCALE * Wq[chsl]).T.astype(np.float16)),
                "wkT": np.ascontiguousarray((SCALE * Wk[chsl]).T.astype(np.float16)),
                "wvT": np.ascontiguousarray(Wv[chsl].T.astype(np.float16)),
                "woT": np.ascontiguousarray(Wo[:, chsl].T.astype(np.float16)),
                "bq": np.ascontiguousarray(SCALE * bq[chsl]),
                "bv": np.ascontiguousarray(bv[chsl]),
                "mpat": mpat,
                "maskT": maskT,
            }
        )

    return nc, in_maps


def kernel(x, mask, Wq, bq, Wk, Wv, bv, Wo, bo):
    nc, in_maps = prepare(x, mask, Wq, bq, Wk, Wv, bv, Wo, bo)
    res = run_bass_kernel_spmd(nc, in_maps, core_ids=list(range(N_CORES)))
    out = np.empty((B, T, S), dtype=np.float32)
    for b in range(B):
        out[b] = res.results[2 * b]["out"] + res.results[2 * b + 1]["out"]
    out += np.asarray(bo, dtype=np.float32)
    return out
